# revision 20
# baseline (speedup 1.0000x reference)
"""GCN layer (gather + segment_sum + linear + relu) on 8 TRN2 NeuronCores.

Strategy (edge-cut partitioning by destination node):
  - Nodes are split into 8 contiguous ranges of 6250; core i owns all edges
    whose dst falls in its range and produces output rows [i*6250,(i+1)*6250).
  - Host sorts each core's edges by dst into 128-node windows. For each
    window the kernel gathers feature[src] rows (fp16 tables, dma_gather with
    int16 indices), builds per-tile one-hot matrices from the local dst ids
    (iota == dst via tensor_scalar on DVE) and accumulates
    h^T[f,n] += msgs^T @ onehot on the TensorEngine in PSUM.
  - Epilogue per window: h^T (PSUM) -> SBUF, out = relu(h @ W + b) via two
    matmuls (W product + rank-1 bias) and a Relu activation, DMA to DRAM.
  - The feature table is replicated per core; gather indices are int16 so the
    table is split in two row-chunks with separate gathers.
"""

import numpy as np

import concourse.bass as bass
import concourse.mybir as mybir
import concourse.tile as tile
from concourse import bacc
from concourse.bass_utils import run_bass_kernel_spmd

P = 128  # partitions / tile edge


class Cfg:
    def __init__(self, n_nodes, n_edges, d, n_cores, dt16=mybir.dt.float16):
        self.n_nodes = n_nodes
        self.n_edges = n_edges
        self.d = d
        self.n_cores = n_cores
        self.nodes_per_core = n_nodes // n_cores
        assert self.nodes_per_core * n_cores == n_nodes
        self.n_windows = (self.nodes_per_core + P - 1) // P
        self.chunk = (n_nodes + 1) // 2  # feature-table row chunks (int16 idx)
        assert self.chunk < 32768
        self.dt16 = dt16
        self.np16 = np.float16 if dt16 == mybir.dt.float16 else np.dtype("bfloat16")


CFG = Cfg(50000, 800000, 128, 8)

PAD_DST = 200.0  # one-hot miss marker (> any local dst, exact in fp16/bf16)


def _prepare(cfg, feature, edge_src, edge_dst, group=4):
    """Host-side shard/sort/pad. Edges keyed (core, win-group, chunk, window)
    so one dma_gather covers a whole (group, chunk) slab. Returns per-core
    arrays + baked tile counts T[ngroups, 2, group]."""
    npc, nw = cfg.nodes_per_core, cfg.n_windows
    ng = (nw + group - 1) // group
    nwp = ng * group  # padded window count
    core = edge_dst // npc
    local = edge_dst - core * npc
    win = local >> 7
    dloc = (local & 127).astype(np.float32)
    chunk = (edge_src >= cfg.chunk).astype(np.int64)
    sloc = (edge_src - chunk * cfg.chunk).astype(np.int16)

    grp = win // group
    wig = win % group
    nkey = (((core * ng + grp) * 2 + chunk) * group + wig).astype(np.int64)
    # ascending src within each block: gather descriptors walk HBM in
    # address order (row-buffer locality; duplicate srcs become adjacent)
    order = np.argsort(nkey * 32768 + sloc, kind="stable")
    sloc, dloc, nkey = sloc[order], dloc[order], nkey[order]
    counts = np.bincount(nkey, minlength=cfg.n_cores * ng * 2 * group).reshape(
        cfg.n_cores, ng, 2, group
    )
    T = (-(-counts // P)).max(axis=0)  # [ng, 2, group]
    rows = T * P
    total_rows = int(rows.sum())
    total_tiles = int(T.sum())
    total_s = total_rows // 16

    starts = np.concatenate([[0], np.cumsum(counts.reshape(-1))])
    blk_off = np.concatenate([[0], np.cumsum(rows.reshape(-1))])
    nblk = ng * 2 * group
    idx16 = np.zeros((cfg.n_cores, 128, total_s), np.int16)
    dstw = np.full((cfg.n_cores, 128, total_tiles), PAD_DST, np.float16)
    for i in range(cfg.n_cores):
        src_pad = np.zeros(total_rows, np.int16)
        dst_pad = np.full(total_rows, PAD_DST, np.float16)
        for bi in range(nblk):
            g = i * nblk + bi
            n = counts.reshape(-1)[g]
            o = blk_off[bi]
            src_pad[o : o + n] = sloc[starts[g] : starts[g] + n]
            dst_pad[o : o + n] = dloc[starts[g] : starts[g] + n]
        # dma_gather idx layout: idx j of a gather sits at [j%16, j//16],
        # replicated across the 8 Q7 cores (16-partition groups).
        idx16[i] = np.tile(src_pad.reshape(total_s, 16).T, (8, 1))
        dstw[i] = dst_pad.reshape(total_tiles, P).T
    return T, idx16, dstw, total_s, total_tiles


def _build(cfg, T, nq=4, repeat=1, mode="full", group=4, mbufs=4):
    """Build the SPMD program. T is the baked [ngroups, 2, group] tile table."""
    f32 = mybir.dt.float32
    dt16 = cfg.dt16
    nw = cfg.n_windows
    ng = T.shape[0]
    total_tiles = int(T.sum())
    total_s = int(T.sum() * P // 16)
    is_equal = mybir.AluOpType.is_equal

    mT = int(T.max())
    nc = bacc.Bacc(None, target_bir_lowering=False, num_swdge_queues=nq)
    feat0 = nc.declare_dram_parameter("feat0", [cfg.chunk, cfg.d], dt16, False)
    feat1 = nc.declare_dram_parameter(
        "feat1", [cfg.n_nodes - cfg.chunk, cfg.d], dt16, False
    )
    idx = nc.declare_dram_parameter("idx16", [P, total_s], mybir.dt.int16, False)
    dstw = nc.declare_dram_parameter("dstw", [P, total_tiles], dt16, False)
    iota = nc.declare_dram_parameter("iota16", [P, mT, P], dt16, False)
    wmat = nc.declare_dram_parameter("wmat", [cfg.d, cfg.d], f32, False)
    bvec = nc.declare_dram_parameter("bvec", [1, cfg.d], f32, False)
    out = nc.declare_dram_parameter("out", [cfg.nodes_per_core, cfg.d], f32, True)

    grp_tiles = T.sum(axis=(1, 2))  # tiles per group
    gt_max = int(grp_tiles.max())

    with tile.TileContext(nc) as tc:
        with (
            tc.tile_pool(name="const", bufs=1) as cpool,
            tc.tile_pool(name="msgs", bufs=mbufs) as mpool,
            tc.tile_pool(name="oh", bufs=4) as ohpool,
            tc.tile_pool(name="ep", bufs=4) as eppool,
            tc.tile_pool(name="psA", bufs=4, space="PSUM") as psa,
            tc.tile_pool(name="psB", bufs=4, space="PSUM") as psb,
        ):
            idx_sb = cpool.tile([P, total_s], mybir.dt.int16)
            nc.sync.dma_start(idx_sb[:], idx[:])
            dst_sb = cpool.tile([P, total_tiles], dt16)
            nc.sync.dma_start(dst_sb[:], dstw[:])
            iota_sb = cpool.tile([P, mT, P], dt16)
            nc.sync.dma_start(iota_sb[:], iota[:])
            w_sb = cpool.tile([cfg.d, cfg.d], f32)
            nc.sync.dma_start(w_sb[:], wmat[:])
            b_sb = cpool.tile([1, cfg.d], f32)
            nc.sync.dma_start(b_sb[:], bvec[:])
            ones_sb = cpool.tile([1, cfg.d], f32)
            nc.vector.memset(ones_sb[:], 1.0)

            if mode != "nogather":
                # first-touch zero of the msgs ring: skipped pad descriptors
                # leave stale SBUF rows, which must not be NaN on first use
                for _i in range(mbufs):
                    mz = mpool.tile([P, gt_max, cfg.d], dt16, tag="msgs")
                    nc.vector.memset(mz[:], 0.0)

            gcount = 0
            for _rep in range(repeat):
                s_off = 0
                t_base = 0
                for g in range(ng):
                    gt_tiles = int(grp_tiles[g])
                    msgs = None
                    if mode != "nogather":
                        msgs = mpool.tile([P, gt_max, cfg.d], dt16, tag="msgs")
                    c_base = [0, int(T[g, 0].sum())]
                    if mode != "nogather":
                        c_off = 0
                        for c in (0, 1):
                            tcg = int(T[g, c].sum())
                            if tcg == 0:
                                continue
                            r = tcg * P
                            nc.gpsimd.dma_gather(
                                msgs[:, c_off : c_off + tcg, :],
                                (feat0 if c == 0 else feat1)[:, :],
                                idx_sb[:, s_off : s_off + r // 16],
                                r,
                                r,
                                cfg.d,
                                single_packet=(r <= 1024),
                                queue_num=gcount % nq,
                            )
                            gcount += 1
                            c_off += tcg
                            s_off += r // 16
                    for wig in range(T.shape[2]):
                        j = g * T.shape[2] + wig
                        if j >= nw:
                            break
                        rows = min(P, cfg.nodes_per_core - j * P)
                        if mode == "nocompute":
                            nc.sync.dma_start(
                                out[j * P : j * P + rows, :], w_sb[:rows, :]
                            )
                            continue
                        tj = int(T[g, 0, wig] + T[g, 1, wig])
                        psum_t = psa.tile([P, P], f32, tag="pT")
                        t = 0
                        for c in (0, 1):
                            ncc = int(T[g, c, wig])
                            if ncc == 0:
                                continue
                            a = c_base[c] + int(T[g, c, :wig].sum())
                            oh = ohpool.tile([P, ncc, P], dt16, tag="oh")
                            nc.vector.tensor_tensor(
                                out=oh[:],
                                in0=iota_sb[:, :ncc, :],
                                in1=dst_sb[
                                    :, t_base + a : t_base + a + ncc
                                ].to_broadcast([P, ncc, P]),
                                op=is_equal,
                            )
                            for i in range(ncc):
                                nc.tensor.matmul(
                                    psum_t[:],
                                    lhsT=(
                                        iota_sb[:, 0, :]
                                        if mode == "nogather"
                                        else msgs[:, a + i, :]
                                    ),
                                    rhs=oh[:, i, :],
                                    start=(t == 0),
                                    stop=(t == tj - 1),
                                )
                                t += 1
                        h_t = eppool.tile([P, P], f32, tag="hT")
                        nc.scalar.activation(
                            h_t[:], psum_t[:], mybir.ActivationFunctionType.Copy
                        )
                        psum_o = psb.tile([P, P], f32, tag="p2")
                        nc.tensor.matmul(
                            psum_o[:], lhsT=h_t[:], rhs=w_sb[:], start=True, stop=False
                        )
                        nc.tensor.matmul(
                            psum_o[:],
                            lhsT=ones_sb[:1, :],
                            rhs=b_sb[:1, :],
                            start=False,
                            stop=True,
                        )
                        ow = eppool.tile([P, P], f32, tag="ow")
                        nc.scalar.activation(
                            ow[:], psum_o[:], mybir.ActivationFunctionType.Relu
                        )
                        nc.sync.dma_start(out[j * P : j * P + rows, :], ow[:rows, :])
                    t_base += gt_tiles
    nc.compile()
    return nc


def make_in_maps(cfg, feature, edge_src, edge_dst, W, b, group=4):
    feature = np.asarray(feature, np.float32)
    edge_src = np.asarray(edge_src, np.int32)
    edge_dst = np.asarray(edge_dst, np.int32)
    W = np.asarray(W, np.float32)
    b = np.asarray(b, np.float32)
    T, idx16, dstw, total_s, total_tiles = _prepare(
        cfg, feature, edge_src, edge_dst, group=group
    )
    f16 = np.ascontiguousarray(feature.astype(cfg.np16))
    feat0, feat1 = f16[: cfg.chunk], f16[cfg.chunk :]
    mT = int(T.max())
    iota16 = np.ascontiguousarray(
        np.broadcast_to(np.arange(P, dtype=np.float32), (P, mT, P))
    ).astype(cfg.np16)
    in_maps = [
        dict(
            feat0=feat0,
            feat1=feat1,
            idx16=np.ascontiguousarray(idx16[i]),
            dstw=np.ascontiguousarray(dstw[i]),
            iota16=iota16,
            wmat=W,
            bvec=b[None, :],
        )
        for i in range(cfg.n_cores)
    ]
    return T, in_maps


_BUILD_CACHE = {}


def run(feature, edge_src, edge_dst, W, b, cfg=CFG, trace=False, nq=4, **spmd_kwargs):
    T, in_maps = make_in_maps(cfg, feature, edge_src, edge_dst, W, b)
    key = (cfg.n_nodes, cfg.n_edges, nq, tuple(T.reshape(-1).tolist()))
    nc = _BUILD_CACHE.get(key)
    if nc is None:
        nc = _build(cfg, T, nq=nq)
        _BUILD_CACHE[key] = nc
    res = run_bass_kernel_spmd(
        nc, in_maps, core_ids=list(range(cfg.n_cores)), trace=trace, **spmd_kwargs
    )
    outs = [np.asarray(res.results[i]["out"]) for i in range(cfg.n_cores)]
    return np.concatenate(outs, axis=0), res


def kernel(**inputs):
    out, _ = run(
        inputs["feature"],
        inputs["edge_src"],
        inputs["edge_dst"],
        inputs["W"],
        inputs["b"],
    )
    return out



# revision 33
# speedup vs baseline: 2.9151x; 2.9151x over previous
"""GCN layer (gather + segment_sum + linear + relu) on 8 TRN2 NeuronCores.

Strategy (edge-cut partitioning by destination node):
  - Nodes are split into 8 contiguous ranges of 6250; core i owns all edges
    whose dst falls in its range and produces output rows [i*6250,(i+1)*6250).
  - Dst nodes are grouped into windows of 104 (< 128: headroom against the
    ceil-to-128-rows tile padding of the per-window edge lists; measured
    faster than 128 on HW). Host sorts each core's edges by (4-window
    group, table chunk, window, src). One dma_gather per (group, chunk)
    slab pulls feature[src] rows (fp16 tables, int16 indices, ascending src
    for HBM row locality) into a 4-deep msgs ring; the gather's per-edge
    256B descriptors are the pipeline bottleneck (~2.4ns/descriptor,
    descriptor-rate-bound and ~linear in SWDGE queue count (max 4) -
    measured identical for SBUF-resident tables, so the table stays in HBM).
  - Per (window, chunk) ONE batched DVE tensor_tensor builds all one-hot
    tiles at once (iota == dst broadcast along a stride-0 axis, fp16 in/out
    for the 2x 16-bit DVE rate); the TensorEngine accumulates
    h^T[f,n] += msgs^T @ onehot in PSUM (fp16, 1 cycle/row).
  - Epilogue per window: h^T PSUM -> SBUF on the otherwise-idle Activation
    engine (Copy), out = relu(h @ W + b) via two matmuls (W product +
    rank-1 bias) and a Relu activation, f32 DMA to DRAM.
  - PSUM pools double-buffered (2/2) + epilogue ring 3 - measured faster
    than deeper buffering on HW.
"""

import numpy as np

import concourse.bass as bass
import concourse.mybir as mybir
import concourse.tile as tile
from concourse import bacc
from concourse.bass_utils import run_bass_kernel_spmd

P = 128  # partitions / tile edge


class Cfg:
    def __init__(self, n_nodes, n_edges, d, n_cores, dt16=mybir.dt.float16, wsize=P):
        self.n_nodes = n_nodes
        self.n_edges = n_edges
        self.d = d
        self.n_cores = n_cores
        self.wsize = wsize  # dst nodes per window (<=128)
        self.nodes_per_core = n_nodes // n_cores
        assert self.nodes_per_core * n_cores == n_nodes
        self.n_windows = (self.nodes_per_core + wsize - 1) // wsize
        self.chunk = (n_nodes + 1) // 2  # feature-table row chunks (int16 idx)
        assert self.chunk < 32768
        self.dt16 = dt16
        self.np16 = np.float16 if dt16 == mybir.dt.float16 else np.dtype("bfloat16")


CFG = Cfg(50000, 800000, 128, 8, wsize=104)

PAD_DST = 200.0  # one-hot miss marker (> any local dst, exact in fp16/bf16)


def _prepare(cfg, feature, edge_src, edge_dst, group=4, negpad=False):
    """Host-side shard/sort/pad. Edges keyed (core, win-group, chunk, window)
    so one dma_gather covers a whole (group, chunk) slab. Returns per-core
    arrays + baked tile counts T[ngroups, 2, group]."""
    npc, nw = cfg.nodes_per_core, cfg.n_windows
    ng = (nw + group - 1) // group
    nwp = ng * group  # padded window count
    core = edge_dst // npc
    local = edge_dst - core * npc
    win = local // cfg.wsize
    dloc = (local % cfg.wsize).astype(np.float32)
    chunk = (edge_src >= cfg.chunk).astype(np.int64)
    sloc = (edge_src - chunk * cfg.chunk).astype(np.int16)

    grp = win // group
    wig = win % group
    nkey = (((core * ng + grp) * 2 + chunk) * group + wig).astype(np.int64)
    # ascending src within each block: gather descriptors walk HBM in
    # address order (row-buffer locality; duplicate srcs become adjacent)
    order = np.argsort(nkey * 32768 + sloc, kind="stable")
    sloc, dloc, nkey = sloc[order], dloc[order], nkey[order]
    counts = np.bincount(nkey, minlength=cfg.n_cores * ng * 2 * group).reshape(
        cfg.n_cores, ng, 2, group
    )
    T = (-(-counts // P)).max(axis=0)  # [ng, 2, group]
    rows = T * P
    total_rows = int(rows.sum())
    total_tiles = int(T.sum())
    total_s = total_rows // 16

    starts = np.concatenate([[0], np.cumsum(counts.reshape(-1))])
    blk_off = np.concatenate([[0], np.cumsum(rows.reshape(-1))])
    nblk = ng * 2 * group
    idx16 = np.zeros((cfg.n_cores, 128, total_s), np.int16)
    dstw = np.full((cfg.n_cores, 128, total_tiles), PAD_DST, np.float16)
    for i in range(cfg.n_cores):
        # negpad: -1 pad slots -> dma_gather emits no descriptor (safe as a
        # trailing run within each per-block gather)
        src_pad = np.full(total_rows, -1 if negpad else 0, np.int16)
        dst_pad = np.full(total_rows, PAD_DST, np.float16)
        for bi in range(nblk):
            g = i * nblk + bi
            n = counts.reshape(-1)[g]
            o = blk_off[bi]
            src_pad[o : o + n] = sloc[starts[g] : starts[g] + n]
            dst_pad[o : o + n] = dloc[starts[g] : starts[g] + n]
        # dma_gather idx layout: idx j of a gather sits at [j%16, j//16],
        # replicated across the 8 Q7 cores (16-partition groups).
        idx16[i] = np.tile(src_pad.reshape(total_s, 16).T, (8, 1))
        dstw[i] = dst_pad.reshape(total_tiles, P).T
    return T, idx16, dstw, total_s, total_tiles


def _build(cfg, T, nq=4, repeat=1, mode="full", group=4, mbufs=4, psbufs=4, epbufs=4, zmsgs=True, per_block=False, f16out=False):
    """Build the SPMD program. T is the baked [ngroups, 2, group] tile table."""
    f32 = mybir.dt.float32
    dt16 = cfg.dt16
    nw = cfg.n_windows
    ng = T.shape[0]
    total_tiles = int(T.sum())
    total_s = int(T.sum() * P // 16)
    is_equal = mybir.AluOpType.is_equal

    mT = int(T.max())
    nc = bacc.Bacc(None, target_bir_lowering=False, num_swdge_queues=nq)
    feat0 = nc.declare_dram_parameter("feat0", [cfg.chunk, cfg.d], dt16, False)
    feat1 = nc.declare_dram_parameter(
        "feat1", [cfg.n_nodes - cfg.chunk, cfg.d], dt16, False
    )
    idx = nc.declare_dram_parameter("idx16", [P, total_s], mybir.dt.int16, False)
    dstw = nc.declare_dram_parameter("dstw", [P, total_tiles], dt16, False)
    iota = nc.declare_dram_parameter("iota16", [P, mT, P], dt16, False)
    wmat = nc.declare_dram_parameter("wmat", [cfg.d, cfg.d], f32, False)
    bvec = nc.declare_dram_parameter("bvec", [1, cfg.d], f32, False)
    odt = dt16 if f16out else f32
    out = nc.declare_dram_parameter("out", [cfg.nodes_per_core, cfg.d], odt, True)

    grp_tiles = T.sum(axis=(1, 2))  # tiles per group
    gt_max = int(grp_tiles.max())

    with tile.TileContext(nc) as tc:
        with (
            tc.tile_pool(name="const", bufs=1) as cpool,
            tc.tile_pool(name="msgs", bufs=mbufs) as mpool,
            tc.tile_pool(name="oh", bufs=4) as ohpool,
            tc.tile_pool(name="ep", bufs=epbufs) as eppool,
            tc.tile_pool(name="psA", bufs=psbufs, space="PSUM") as psa,
            tc.tile_pool(name="psB", bufs=psbufs, space="PSUM") as psb,
        ):
            idx_sb = cpool.tile([P, total_s], mybir.dt.int16)
            nc.sync.dma_start(idx_sb[:], idx[:])
            dst_sb = cpool.tile([P, total_tiles], dt16)
            nc.sync.dma_start(dst_sb[:], dstw[:])
            iota_sb = cpool.tile([P, mT, P], dt16)
            nc.sync.dma_start(iota_sb[:], iota[:])
            w_sb = cpool.tile([cfg.d, cfg.d], f32)
            nc.sync.dma_start(w_sb[:], wmat[:])
            b_sb = cpool.tile([1, cfg.d], f32)
            nc.sync.dma_start(b_sb[:], bvec[:])
            ones_sb = cpool.tile([1, cfg.d], f32)
            nc.vector.memset(ones_sb[:], 1.0)

            if zmsgs and mode != "nogather":
                # first-touch zero of the msgs ring: skipped pad descriptors
                # leave stale SBUF rows, which must not be NaN on first use
                for _i in range(mbufs):
                    mz = mpool.tile([P, gt_max, cfg.d], dt16, tag="msgs")
                    nc.vector.memset(mz[:], 0.0)

            gcount = 0
            for _rep in range(repeat):
                s_off = 0
                t_base = 0
                for g in range(ng):
                    gt_tiles = int(grp_tiles[g])
                    msgs = None
                    if mode != "nogather":
                        msgs = mpool.tile([P, gt_max, cfg.d], dt16, tag="msgs")
                    c_base = [0, int(T[g, 0].sum())]
                    if mode != "nogather":
                        c_off = 0
                        for c in (0, 1):
                            blocks = (
                                [int(T[g, c, w]) for w in range(T.shape[2])]
                                if per_block
                                else [int(T[g, c].sum())]
                            )
                            for tcg in blocks:
                                if tcg == 0:
                                    continue
                                r = tcg * P
                                nc.gpsimd.dma_gather(
                                    msgs[:, c_off : c_off + tcg, :],
                                    (feat0 if c == 0 else feat1)[:, :],
                                    idx_sb[:, s_off : s_off + r // 16],
                                    r,
                                    r,
                                    cfg.d,
                                    single_packet=(r <= 1024),
                                    queue_num=gcount % nq,
                                )
                                gcount += 1
                                c_off += tcg
                                s_off += r // 16
                    for wig in range(T.shape[2]):
                        j = g * T.shape[2] + wig
                        if j >= nw:
                            break
                        W = cfg.wsize
                        rows = min(W, cfg.nodes_per_core - j * W)
                        if mode == "nocompute":
                            nc.sync.dma_start(
                                out[j * W : j * W + rows, :],
                                iota_sb[:rows, 0, :]
                                if f16out
                                else w_sb[:rows, :],
                            )
                            continue
                        tj = int(T[g, 0, wig] + T[g, 1, wig])
                        psum_t = psa.tile([P, P], f32, tag="pT")
                        t = 0
                        for c in (0, 1):
                            ncc = int(T[g, c, wig])
                            if ncc == 0:
                                continue
                            a = c_base[c] + int(T[g, c, :wig].sum())
                            oh = ohpool.tile([P, ncc, P], dt16, tag="oh")
                            nc.vector.tensor_tensor(
                                out=oh[:],
                                in0=iota_sb[:, :ncc, :],
                                in1=dst_sb[
                                    :, t_base + a : t_base + a + ncc
                                ].to_broadcast([P, ncc, P]),
                                op=is_equal,
                            )
                            for i in range(ncc):
                                nc.tensor.matmul(
                                    psum_t[:],
                                    lhsT=(
                                        iota_sb[:, 0, :]
                                        if mode == "nogather"
                                        else msgs[:, a + i, :]
                                    ),
                                    rhs=oh[:, i, :],
                                    start=(t == 0),
                                    stop=(t == tj - 1),
                                )
                                t += 1
                        h_t = eppool.tile([P, P], f32, tag="hT")
                        nc.scalar.activation(
                            h_t[:], psum_t[:], mybir.ActivationFunctionType.Copy
                        )
                        psum_o = psb.tile([P, P], f32, tag="p2")
                        nc.tensor.matmul(
                            psum_o[:], lhsT=h_t[:], rhs=w_sb[:], start=True, stop=False
                        )
                        nc.tensor.matmul(
                            psum_o[:],
                            lhsT=ones_sb[:1, :],
                            rhs=b_sb[:1, :],
                            start=False,
                            stop=True,
                        )
                        ow = eppool.tile([P, P], odt, tag="ow")
                        nc.scalar.activation(
                            ow[:], psum_o[:], mybir.ActivationFunctionType.Relu
                        )
                        nc.sync.dma_start(out[j * W : j * W + rows, :], ow[:rows, :])
                    t_base += gt_tiles
    nc.compile()
    return nc


def make_in_maps(cfg, feature, edge_src, edge_dst, W, b, group=4, negpad=False):
    feature = np.asarray(feature, np.float32)
    edge_src = np.asarray(edge_src, np.int32)
    edge_dst = np.asarray(edge_dst, np.int32)
    W = np.asarray(W, np.float32)
    b = np.asarray(b, np.float32)
    T, idx16, dstw, total_s, total_tiles = _prepare(
        cfg, feature, edge_src, edge_dst, group=group, negpad=negpad
    )
    f16 = np.ascontiguousarray(feature.astype(cfg.np16))
    feat0, feat1 = f16[: cfg.chunk], f16[cfg.chunk :]
    mT = int(T.max())
    iota16 = np.ascontiguousarray(
        np.broadcast_to(np.arange(P, dtype=np.float32), (P, mT, P))
    ).astype(cfg.np16)
    in_maps = [
        dict(
            feat0=feat0,
            feat1=feat1,
            idx16=np.ascontiguousarray(idx16[i]),
            dstw=np.ascontiguousarray(dstw[i]),
            iota16=iota16,
            wmat=W,
            bvec=b[None, :],
        )
        for i in range(cfg.n_cores)
    ]
    return T, in_maps


_BUILD_CACHE = {}

# winning config from interleaved A/B on HW: psum/ep double-buffering at
# 2/3 beats 4/4; msgs pre-zero only needed with negpad (skipped descriptors)
BEST = dict(psbufs=2, epbufs=3, zmsgs=False, mbufs=4, per_block=False, f16out=False)


def run(feature, edge_src, edge_dst, W, b, cfg=CFG, trace=False, nq=4, **spmd_kwargs):
    T, in_maps = make_in_maps(
        cfg, feature, edge_src, edge_dst, W, b, negpad=BEST["per_block"]
    )
    key = (cfg.n_nodes, cfg.n_edges, nq, tuple(T.reshape(-1).tolist()))
    nc = _BUILD_CACHE.get(key)
    if nc is None:
        nc = _build(cfg, T, nq=nq, **BEST)
        _BUILD_CACHE[key] = nc
    res = run_bass_kernel_spmd(
        nc, in_maps, core_ids=list(range(cfg.n_cores)), trace=trace, **spmd_kwargs
    )
    outs = [np.asarray(res.results[i]["out"]) for i in range(cfg.n_cores)]
    return np.concatenate(outs, axis=0).astype(np.float32), res


def kernel(**inputs):
    out, _ = run(
        inputs["feature"],
        inputs["edge_src"],
        inputs["edge_dst"],
        inputs["W"],
        inputs["b"],
    )
    return out

